# revision 1
# baseline (speedup 1.0000x reference)
"""Trainium2 Bass kernel for nn_MultiHeadAttention_36051955483000.

Full-shape contract: kernel(**inputs) takes the complete fp32 tensors
(q,k,v: [4,2048,1024]; Wq/Wk/Wv/Wo: [1024,1024]; biases [1024]) and
returns the full [4,2048,1024] fp32 output.

Sharding (8 NeuronCores): core = 2*b + g for batch b in 0..3 and
head-group g in {0,1}. Each core computes 8 of the 16 heads for one
batch: Q/K/V projections with the 512-column weight slice, causal
attention, a pairwise AllGather of the attention output across the two
head-group cores of a batch, then the output projection for its 512
output features. Host-side work is limited to dtype casts, transposes,
and concatenation.

Kernel structure notes:
- All matmuls bf16 with fp32 PSUM accumulation.
- Scores are computed transposed (S^T: keys on partitions) so exp(S^T)
  feeds the P@V matmul directly as the stationary operand's transpose,
  with no on-chip transposes of P.
- No softmax max-subtraction: the causal mask adds -32768 before the
  fused exp(0.125*x), which underflows to exactly 0 in fp32.
- Softmax denominators come from an all-ones column appended to V
  (per head), so they fall out of the same PE accumulation.
- Heads are processed in pairs on disjoint PE row-groups (contraction
  is only 64 deep), doubling score-matmul throughput and letting
  LDWEIGHTS overlap matmuls.
- Emission is software-pipelined: the PV matmuls of score-group i are
  emitted after the scores+exp of group i+1, so the tensor engine
  always has independent work while the scalar engine runs exp.
- The AllGather is chunked per 128-feature block and overlapped with
  attention of the remaining heads; Wo^T rows are host-permuted to
  match the chunked gather's block order.
"""

import numpy as np
import ml_dtypes

B, N, D, H = 4, 2048, 1024, 16
DH = D // H            # 64
HG = H // 2            # 8 heads per core
FG = D // 2            # 512 features per head-group
N_CORES = 8
QC = 256               # query-chunk width
NQB = N // 128         # 16 query blocks
NKB = N // 128         # 16 key blocks
MASK = -32768.0        # exp(0.125 * (s + MASK)) == 0 exactly in fp32

BF16 = ml_dtypes.bfloat16
# chunked-AllGather feature-block order (see _build_program)
PERM = [0, 4, 1, 5, 2, 6, 3, 7]

_PROG = None


def _build_program():
    from concourse import bacc, tile, mybir

    f32 = mybir.dt.float32
    bf16 = mybir.dt.bfloat16

    nc = bacc.Bacc("TRN2", target_bir_lowering=False, debug=False,
                   num_devices=N_CORES)

    xqT = nc.dram_tensor("xqT", [D, N], bf16, kind="ExternalInput").ap()
    xkT = nc.dram_tensor("xkT", [D, N], bf16, kind="ExternalInput").ap()
    xvT = nc.dram_tensor("xvT", [D, N], bf16, kind="ExternalInput").ap()
    wqT = nc.dram_tensor("wqT", [D, FG], bf16, kind="ExternalInput").ap()
    wkT = nc.dram_tensor("wkT", [D, FG], bf16, kind="ExternalInput").ap()
    wvT = nc.dram_tensor("wvT", [D, FG], bf16, kind="ExternalInput").ap()
    woT = nc.dram_tensor("woT", [D, FG], bf16, kind="ExternalInput").ap()
    bq2 = nc.dram_tensor("bq2", [128, 4], f32, kind="ExternalInput").ap()
    bk2 = nc.dram_tensor("bk2", [128, 4], f32, kind="ExternalInput").ap()
    tri01 = nc.dram_tensor("tri01", [128, 128], bf16, kind="ExternalInput").ap()
    y = nc.dram_tensor("y", [N, FG], f32, kind="ExternalOutput").ap()

    add = mybir.AluOpType.add
    mult = mybir.AluOpType.mult
    Exp = mybir.ActivationFunctionType.Exp

    with tile.TileContext(nc) as tc:
        with (
            tc.tile_pool(name="consts", bufs=1) as consts,
            tc.tile_pool(name="dram", bufs=1, space="DRAM") as dram,
            tc.tile_pool(name="xin", bufs=8) as xin,
        ):
            wq_sb = consts.tile([128, 8 * FG], bf16, tag="wq")
            wk_sb = consts.tile([128, 8 * FG], bf16, tag="wk")
            wv_sb = consts.tile([128, 8 * FG], bf16, tag="wv")
            wo_sb = consts.tile([128, 8 * FG], bf16, tag="wo")
            qt_sb = consts.tile([128, 4 * N], bf16, tag="qt")
            kt_sb = consts.tile([128, 4 * N], bf16, tag="kt")
            vaug_sb = consts.tile([128, NKB * HG * 65], bf16, tag="vaug")
            xtown = consts.tile([128, 4 * N], bf16, tag="xtown")
            ones_sb = consts.tile([128, 64], f32, tag="ones")
            bq_sb = consts.tile([128, 4], f32, tag="bq")
            bk_sb = consts.tile([128, 4], f32, tag="bk")
            tri_sb = consts.tile([128, 128], bf16, tag="tri")

            cc_in = [[dram.tile([128, N // 2], bf16, name=f"cc_in{e}_{t}",
                               tag=f"cci{e}_{t}") for t in range(2)]
                     for e in range(4)]
            cc_out = [[dram.tile([256, N // 2], bf16, name=f"cc_out{e}_{t}",
                                tag=f"cco{e}_{t}") for t in range(2)]
                      for e in range(4)]

            # small consts on the sync queue; weights on the scalar queue
            nc.sync.dma_start(bq_sb[:], bq2[:])
            nc.sync.dma_start(bk_sb[:], bk2[:])
            nc.sync.dma_start(tri_sb[:], tri01[:])
            for db in range(8):
                nc.scalar.dma_start(wq_sb[:, 512 * db:512 * db + 512],
                                    wqT[128 * db:128 * db + 128, :])
            for db in range(8):
                nc.scalar.dma_start(wk_sb[:, 512 * db:512 * db + 512],
                                    wkT[128 * db:128 * db + 128, :])
            for db in range(8):
                nc.scalar.dma_start(wv_sb[:, 512 * db:512 * db + 512],
                                    wvT[128 * db:128 * db + 128, :])
            for db in range(8):
                nc.scalar.dma_start(wo_sb[:, 512 * db:512 * db + 512],
                                    woT[128 * db:128 * db + 128, :])

            vaug_v = vaug_sb[:, :].rearrange("p (t h c) -> p t h c",
                                             t=NKB, h=HG, c=65)
            nc.vector.memset(vaug_v[:, :, :, 64:65], 1.0)
            nc.vector.memset(ones_sb[:, :], 1.0)

            # ---- projections ----
            with tc.tile_pool(name="pp", bufs=4, space="PSUM") as pp:
                for XT, W_sb, OUT_sb, bias, nm in (
                    (xqT, wq_sb, qt_sb, bq_sb, "xq"),
                    (xkT, wk_sb, kt_sb, bk_sb, "xk"),
                ):
                    xts = [xin.tile([128, N], bf16, tag="xin", name=f"{nm}{db}")
                           for db in range(8)]
                    for db in range(8):
                        nc.sync.dma_start(xts[db][:],
                                          XT[128 * db:128 * db + 128, :])
                    for tcx in range(4):
                        pss = [pp.tile([128, 512], f32, tag="projp",
                                       name=f"projp{fb}") for fb in range(4)]
                        for db in range(8):
                            for fb in range(4):
                                nc.tensor.matmul(
                                    pss[fb][:],
                                    lhsT=W_sb[:, 512 * db + 128 * fb:
                                              512 * db + 128 * fb + 128],
                                    rhs=xts[db][:, 512 * tcx:512 * tcx + 512],
                                    start=(db == 0), stop=(db == 7))
                        for fb in range(4):
                            nc.vector.tensor_scalar(
                                OUT_sb[:, 2048 * fb + 512 * tcx:
                                       2048 * fb + 512 * tcx + 512],
                                pss[fb][:], bias[:, fb:fb + 1], None, add)
                # V
                xvs = [xin.tile([128, N], bf16, tag="xin", name=f"xv{db}")
                       for db in range(8)]
                for db in range(8):
                    nc.sync.dma_start(xvs[db][:],
                                      xvT[128 * db:128 * db + 128, :])
                for tb in range(NKB):
                    ps = pp.tile([128, 512], f32, tag="projp", name="projpv")
                    for db in range(8):
                        nc.tensor.matmul(
                            ps[:],
                            lhsT=xvs[db][:, 128 * tb:128 * tb + 128],
                            rhs=wv_sb[:, 512 * db:512 * db + 512],
                            start=(db == 0), stop=(db == 7))
                    nc.vector.tensor_copy(
                        vaug_v[:, tb, :, 0:64],
                        ps[:, :].rearrange("p (h c) -> p h c", h=HG, c=64))

            # ---- attention (head pairs on disjoint PE row groups) ----
            with (
                tc.tile_pool(name="sg", bufs=2, space="PSUM") as sgp,
                tc.tile_pool(name="otp", bufs=3, space="PSUM") as otp,
                tc.tile_pool(name="bcp", bufs=1, space="PSUM") as bcp,
                tc.tile_pool(name="pt", bufs=3) as ptp,
                tc.tile_pool(name="ep", bufs=4) as ep,
            ):
                for e in range(4):
                    hb = 2048 * e

                    def emit_epilogue(OT2, c):
                        # OT2 is (65, 512): rows 0:64 = O^T for the two heads
                        # (head 2e cols 0:256, head 2e+1 cols 256:512), row 64
                        # = softmax denominators. Normalize and write x^T.
                        dn = ep.tile([128, 2 * QC], f32, tag="dn",
                                     name=f"dn{e}_{c}")
                        nc.vector.tensor_copy(dn[64:65, :], OT2[64:65, :])
                        bc = bcp.tile([64, 2 * QC], f32, tag="bc",
                                      name=f"bc{e}_{c}")
                        nc.tensor.matmul(bc[:, :], lhsT=ones_sb[64:65, :],
                                         rhs=dn[64:65, :], start=True, stop=True)
                        bcs = ep.tile([64, 2 * QC], f32, tag="bcs",
                                      name=f"bcs{e}_{c}")
                        nc.vector.reciprocal_approx_fast(bcs[:, :], bc[:, :])
                        for half in (0, 1):
                            nc.vector.tensor_tensor(
                                xtown[64 * half:64 * half + 64,
                                      hb + QC * c:hb + QC * c + QC],
                                OT2[0:64, QC * half:QC * half + QC],
                                bcs[:, QC * half:QC * half + QC], mult)

                    # stream of score-groups: per chunk c, groups of 2 kblocks
                    stream = []
                    for c in range(8):
                        ngroups = c + 1
                        for gi in range(ngroups):
                            stream.append((c, [2 * gi, 2 * gi + 1],
                                           gi == 0, gi == ngroups - 1))
                    ots_by_chunk = {}
                    prev = None
                    for item in stream + [None]:
                        cur = None
                        if item is not None:
                            c, js, first, last = item
                            if first:
                                OT2 = otp.tile([65, 2 * QC], f32, tag="OT2",
                                               name=f"OT2{e}_{c}")
                                ots_by_chunk[c] = OT2
                            SG = sgp.tile([128, 4 * QC], f32, tag="SG",
                                          name=f"SG{e}_{c}_{js[0]}")
                            for m, j in enumerate(js):
                                for half in (0, 1):
                                    po = 64 * half
                                    off = 512 * half + QC * m
                                    kt_j = kt_sb[po:po + 64,
                                                 hb + 128 * j:hb + 128 * j + 128]
                                    if j <= 2 * c:
                                        nc.tensor.matmul(
                                            SG[:, off:off + QC], lhsT=kt_j,
                                            rhs=qt_sb[po:po + 64,
                                                      hb + QC * c:hb + QC * c + QC],
                                            start=True, stop=True,
                                            skip_group_check=True)
                                    else:  # j == 2c+1: front half is dead
                                        nc.tensor.matmul(
                                            SG[:, off + 128:off + QC],
                                            lhsT=kt_j,
                                            rhs=qt_sb[po:po + 64,
                                                      hb + QC * c + 128:
                                                      hb + QC * c + QC],
                                            start=True, stop=True,
                                            skip_group_check=True)
                            PT = ptp.tile([128, 4 * QC], bf16, tag="PT",
                                          name=f"PT{e}_{c}_{js[0]}")
                            nc.scalar.activation(PT[:, :], SG[:, :], Exp,
                                                 scale=0.125)
                            if js[-1] == 2 * c + 1:  # band group: mask on PT
                                for half in (0, 1):
                                    off = 512 * half
                                    # diag block of j=2c (queries 0:128)
                                    nc.vector.tensor_tensor(
                                        PT[:, off:off + 128],
                                        PT[:, off:off + 128], tri_sb[:], mult)
                                    # j=2c+1: dead front half, diag back half
                                    nc.vector.memset(
                                        PT[:, off + QC:off + QC + 128], 0.0)
                                    nc.vector.tensor_tensor(
                                        PT[:, off + QC + 128:off + 2 * QC],
                                        PT[:, off + QC + 128:off + 2 * QC],
                                        tri_sb[:], mult)
                            cur = (c, js, PT)
                        if prev is not None:
                            pc, pjs, pPT = prev
                            pOT2 = ots_by_chunk[pc]
                            for m, j in enumerate(pjs):
                                for half in (0, 1):
                                    nc.tensor.matmul(
                                        pOT2[:, QC * half:QC * half + QC],
                                        lhsT=vaug_sb[:, 65 * HG * j +
                                                     65 * (2 * e + half):
                                                     65 * HG * j +
                                                     65 * (2 * e + half) + 65],
                                        rhs=pPT[:, 512 * half + QC * m:
                                                512 * half + QC * m + QC],
                                        # one start per PSUM bank: start=True
                                        # clears has_written bank-wide, so only
                                        # the tile's first matmul may carry it
                                        start=(j == 0 and half == 0),
                                        stop=(j == 2 * pc + 1),
                                        skip_group_check=True)
                            if pjs[-1] == 2 * pc + 1:  # chunk pc complete
                                emit_epilogue(pOT2, pc)
                                del ots_by_chunk[pc]
                                if pc in (3, 7):
                                    # half the tokens of feature block e done:
                                    # kick that half's pairwise AllGather
                                    t = pc // 4
                                    nc.sync.dma_start(
                                        cc_in[e][t][:],
                                        xtown[:, hb + 1024 * t:
                                              hb + 1024 * t + 1024])
                                    nc.gpsimd.collective_compute(
                                        "AllGather",
                                        mybir.AluOpType.bypass,
                                        replica_groups=[[0, 1], [2, 3],
                                                        [4, 5], [6, 7]],
                                        ins=[cc_in[e][t].opt()],
                                        outs=[cc_out[e][t].opt()],
                                    )
                        prev = cur


            # ---- output projection: y_half = x @ Wo_half^T ----
            # gathered block order: cc_out[e] rows = global feature blocks
            # [e, 4+e]; Wo^T rows are host-permuted to PERM to match.
            with (
                tc.tile_pool(name="opp", bufs=2, space="PSUM") as opp,
                tc.tile_pool(name="yp", bufs=2) as yp,
            ):
                xts = []
                for ci in range(4):
                    for r2 in range(2):
                        xt = xin.tile([128, N], bf16, tag="xin",
                                      name=f"xt{ci}_{r2}")
                        for t in range(2):
                            nc.sync.dma_start(
                                xt[:, 1024 * t:1024 * t + 1024],
                                cc_out[ci][t][128 * r2:128 * r2 + 128, :])
                        xts.append(xt)
                for tb in range(NQB):
                    ps = opp.tile([128, 512], f32, tag="ops", name="ops")
                    for dbp in range(8):
                        nc.tensor.matmul(
                            ps[:],
                            lhsT=xts[dbp][:, 128 * tb:128 * tb + 128],
                            rhs=wo_sb[:, 512 * dbp:512 * dbp + 512],
                            start=(dbp == 0), stop=(dbp == 7))
                    ysb = yp.tile([128, 512], f32, tag="ysb", name="ysb")
                    nc.vector.tensor_copy(ysb[:], ps[:])
                    nc.sync.dma_start(y[128 * tb:128 * tb + 128, :], ysb[:])

    nc.compile()
    return nc


def _program():
    global _PROG
    if _PROG is None:
        _PROG = _build_program()
    return _PROG


def _host_inputs(q, k, v, Wq, bq, Wk, bk, Wv, bv, Wo):
    qb = np.asarray(q, np.float32).astype(BF16)
    kb = np.asarray(k, np.float32).astype(BF16)
    vb = np.asarray(v, np.float32).astype(BF16)
    xqT = [np.ascontiguousarray(qb[b].T) for b in range(B)]
    xkT = [np.ascontiguousarray(kb[b].T) for b in range(B)]
    xvT = [np.ascontiguousarray(vb[b].T) for b in range(B)]

    def wslice(W, g):
        return np.ascontiguousarray(
            np.asarray(W, np.float32)[FG * g:FG * (g + 1), :].T).astype(BF16)

    wqg = [wslice(Wq, g) for g in range(2)]
    wkg = [wslice(Wk, g) for g in range(2)]
    wvg = [wslice(Wv, g) for g in range(2)]

    def woslice(g):
        wt = np.ascontiguousarray(
            np.asarray(Wo, np.float32)[FG * g:FG * (g + 1), :].T).astype(BF16)
        # permute 128-row input-feature blocks to the chunked-AG order
        return np.ascontiguousarray(
            wt.reshape(8, 128, FG)[PERM].reshape(D, FG))

    wog = [woslice(g) for g in range(2)]

    def bslice(bvec, g):
        return np.ascontiguousarray(
            np.asarray(bvec, np.float32)[FG * g:FG * (g + 1)]
            .reshape(4, 128).T)

    bqg = [bslice(bq, g) for g in range(2)]
    bkg = [bslice(bk, g) for g in range(2)]

    kk, qq = np.meshgrid(np.arange(128), np.arange(128), indexing="ij")
    tri = np.where(kk <= qq, 1.0, 0.0).astype(BF16)

    in_maps = []
    for core in range(N_CORES):
        b, g = core // 2, core % 2
        in_maps.append({
            "xqT": xqT[b], "xkT": xkT[b], "xvT": xvT[b],
            "wqT": wqg[g], "wkT": wkg[g], "wvT": wvg[g], "woT": wog[g],
            "bq2": bqg[g], "bk2": bkg[g], "tri01": tri,
        })
    return in_maps


def run_sharded(in_maps, trace=False, trace_kwargs=None):
    from concourse.bass_utils import run_bass_kernel_spmd
    nc = _program()
    return run_bass_kernel_spmd(nc, in_maps, core_ids=list(range(N_CORES)),
                                trace=trace, trace_kwargs=trace_kwargs or {})


def kernel(q, k, v, Wq, bq, Wk, bk, Wv, bv, Wo):
    in_maps = _host_inputs(q, k, v, Wq, bq, Wk, bk, Wv, bv, Wo)
    res = run_sharded(in_maps)
    out = np.empty((B, N, D), np.float32)
    for b in range(B):
        out[b, :, 0:FG] = res.results[2 * b]["y"]
        out[b, :, FG:D] = res.results[2 * b + 1]["y"]
    return out



# revision 21
# speedup vs baseline: 1.1098x; 1.1098x over previous
"""Trainium2 Bass kernel for nn_MultiHeadAttention_36051955483000.

Full-shape contract: kernel(**inputs) takes the complete fp32 tensors
(q,k,v: [4,2048,1024]; Wq/Wk/Wv/Wo: [1024,1024]; biases [1024]) and
returns the full [4,2048,1024] fp32 output.

Sharding (8 NeuronCores): core = 2*b + g for batch b in 0..3 and
head-group g in {0,1}. Each core computes 8 of the 16 heads for one
batch: Q/K/V projections with the 512-column weight slice, causal
attention, a pairwise AllGather of the attention output across the two
head-group cores of a batch, then the output projection for its 512
output features.

v2 design (single fused instruction stream):
- PE warmup matmuls at t=0 so the HAM clock gate is released before
  real work arrives (else first ~40us run at half clock).
- Input DMA order xq -> xv -> xk across parallel queue rings so the
  tensor engine has projection work from ~12us.
- The scalar engine (softmax exp over ~19M elements) is the attention
  bottleneck, so all projection matmuls (Q/K/V for later head-pairs
  and the output projection) are woven as FILLER between attention
  score-groups: the tensor engine works through projections while the
  scalar engine chews exp.
- V is augmented with 64 ones-columns per (kblock, head) so the PV
  matmul (M=128) emits softmax denominators replicated across 64 PSUM
  partitions: the epilogue is one reciprocal + two multiplies, no
  1-partition copies and no broadcast matmul.
- Scores are computed transposed (S^T) with head pairs on disjoint PE
  row groups (concurrent K=64 matmuls); band-group dead halves are
  never streamed through PV and never memset.
- AllGather chunked per (head-pair, token-half), overlapped with
  attention; output projection is split by token half: the t=0 half
  runs as filler inside the last head-pair block, only the t=1 half
  (~8 matmul units) trails the final collective.
"""

import numpy as np
import ml_dtypes

B, N, D, H = 4, 2048, 1024, 16
DH = D // H            # 64
HG = H // 2            # 8 heads per core
FG = D // 2            # 512 features per head-group
N_CORES = 8
QC = 256               # query-chunk width
NKB = N // 128         # 16 key blocks
NQB = N // 128         # 16 token blocks

BF16 = ml_dtypes.bfloat16
# chunked-AllGather feature-block order (see _build_program)
PERM = [0, 4, 1, 5, 2, 6, 3, 7]

WARMUP_MMS = 44

_PROG = None


def _build_program():
    from concourse import bacc, tile, mybir

    f32 = mybir.dt.float32
    bf16 = mybir.dt.bfloat16

    nc = bacc.Bacc("TRN2", target_bir_lowering=False, debug=False,
                   num_devices=N_CORES)

    xqT = nc.dram_tensor("xqT", [D, N], bf16, kind="ExternalInput").ap()
    xkT = nc.dram_tensor("xkT", [D, N], bf16, kind="ExternalInput").ap()
    xvT = nc.dram_tensor("xvT", [D, N], bf16, kind="ExternalInput").ap()
    wqT = nc.dram_tensor("wqT", [D, FG], bf16, kind="ExternalInput").ap()
    wkT = nc.dram_tensor("wkT", [D, FG], bf16, kind="ExternalInput").ap()
    wvT = nc.dram_tensor("wvT", [D, FG], bf16, kind="ExternalInput").ap()
    woT = nc.dram_tensor("woT", [D, FG], bf16, kind="ExternalInput").ap()
    bq2 = nc.dram_tensor("bq2", [128, 4], f32, kind="ExternalInput").ap()
    bk2 = nc.dram_tensor("bk2", [128, 4], f32, kind="ExternalInput").ap()
    tri01 = nc.dram_tensor("tri01", [128, 128], bf16, kind="ExternalInput").ap()
    y = nc.dram_tensor("y", [N, FG], f32, kind="ExternalOutput").ap()
    dbg = nc.dram_tensor("dbg", [128, 1024], bf16, kind="ExternalOutput").ap()

    add = mybir.AluOpType.add
    mult = mybir.AluOpType.mult
    Exp = mybir.ActivationFunctionType.Exp

    with tile.TileContext(nc) as tc:
        with (
            tc.tile_pool(name="consts", bufs=1) as consts,
            tc.tile_pool(name="dram", bufs=1, space="DRAM") as dram,
            tc.tile_pool(name="xin", bufs=24) as xin,
            tc.tile_pool(name="pt", bufs=2) as ptp,
            tc.tile_pool(name="bcs", bufs=2) as bcsp,
            tc.tile_pool(name="ysb", bufs=1) as ysbp,
            tc.tile_pool(name="sg", bufs=2, space="PSUM") as sgp,
            tc.tile_pool(name="ot", bufs=2, space="PSUM") as otp,
            tc.tile_pool(name="pp", bufs=2, space="PSUM") as pp,
        ):
            wq_sb = consts.tile([128, 8 * FG], bf16, tag="wq")
            wk_sb = consts.tile([128, 8 * FG], bf16, tag="wk")
            wv_sb = consts.tile([128, 8 * FG], bf16, tag="wv")
            wo_sb = consts.tile([128, 8 * FG], bf16, tag="wo")
            qt_sb = consts.tile([128, 4 * N], bf16, tag="qt")
            kt_sb = consts.tile([128, 4 * N], bf16, tag="kt")
            # per (kblock, head): cols 0:64 = V^T block, 64:128 = ones
            vaug = consts.tile([128, NKB * HG * 128], bf16, tag="vaug")
            xtown = consts.tile([128, 2048], bf16, tag="xtown")
            warm_r = consts.tile([128, 512], bf16, tag="warm")
            bq_sb = consts.tile([128, 4], f32, tag="bq")
            bk_sb = consts.tile([128, 4], f32, tag="bk")
            tri_sb = consts.tile([128, 128], bf16, tag="tri")

            vaug_v = vaug[:, :].rearrange("p (t h c) -> p t h c",
                                          t=NKB, h=HG, c=128)

            cc_in = [[dram.tile([128, N // 2], bf16, name=f"cc_in{e}_{t}",
                                tag=f"cci{e}_{t}") for t in range(2)]
                     for e in range(4)]
            cc_out = [[dram.tile([256, N // 2], bf16, name=f"cc_out{e}_{t}",
                                 tag=f"cco{e}_{t}") for t in range(2)]
                      for e in range(4)]

            # ---- DMA triggers ----
            # gpsimd queue: small consts, wv early (VU needs it ~33us);
            # wo is deferred into the e0 stream (needed only at e3).
            nc.gpsimd.dma_start(bq_sb[:], bq2[:])
            nc.gpsimd.dma_start(bk_sb[:], bk2[:])
            nc.gpsimd.dma_start(tri_sb[:], tri01[:])
            for db in range(8):
                nc.gpsimd.dma_start(wv_sb[:, 512 * db:512 * db + 512],
                                    wvT[128 * db:128 * db + 128, :])
            # scalar queue (idle until exp starts): wq, wk
            for db in range(8):
                nc.scalar.dma_start(wq_sb[:, 512 * db:512 * db + 512],
                                    wqT[128 * db:128 * db + 128, :])
            for db in range(8):
                nc.scalar.dma_start(wk_sb[:, 512 * db:512 * db + 512],
                                    wkT[128 * db:128 * db + 128, :])
            # sync queue: the big x stream in need-order xq, xv, xk
            xq = [xin.tile([128, N], bf16, tag="xin", name=f"xq{db}")
                  for db in range(8)]
            for db in range(8):
                nc.sync.dma_start(xq[db][:], xqT[128 * db:128 * db + 128, :])
            xv = [xin.tile([128, N], bf16, tag="xin", name=f"xv{db}")
                  for db in range(8)]
            for db in range(8):
                nc.sync.dma_start(xv[db][:], xvT[128 * db:128 * db + 128, :])
            xk = [xin.tile([128, N], bf16, tag="xin", name=f"xk{db}")
                  for db in range(8)]
            for db in range(8):
                nc.sync.dma_start(xk[db][:], xkT[128 * db:128 * db + 128, :])

            nc.vector.memset(warm_r[:, :], 1.0)
            nc.vector.memset(vaug_v[:, :, :, 64:128], 1.0)

            # ---- PE warmup: release the HAM clock gate before real work
            wps = pp.tile([128, 512], f32, tag="pp", name="warm")
            for _ in range(WARMUP_MMS):
                nc.tensor.matmul(wps[:], lhsT=warm_r[:, 0:128], rhs=warm_r[:],
                                 start=True, stop=True)

            # ---- unit emitters (each ~0.9-1.7us of tensor work) ----
            def QU(e, tcx):
                ps = pp.tile([128, 512], f32, tag="pp", name=f"q{e}{tcx}")
                for db in range(8):
                    nc.tensor.matmul(
                        ps[:],
                        lhsT=wq_sb[:, 512 * db + 128 * e:512 * db + 128 * e + 128],
                        rhs=xq[db][:, 512 * tcx:512 * tcx + 512],
                        start=(db == 0), stop=(db == 7))
                nc.vector.tensor_scalar(
                    qt_sb[:, 2048 * e + 512 * tcx:2048 * e + 512 * tcx + 512],
                    ps[:], bq_sb[:, e:e + 1], None, add)

            def KU(e, tcx):
                ps = pp.tile([128, 512], f32, tag="pp", name=f"k{e}{tcx}")
                for db in range(8):
                    nc.tensor.matmul(
                        ps[:],
                        lhsT=wk_sb[:, 512 * db + 128 * e:512 * db + 128 * e + 128],
                        rhs=xk[db][:, 512 * tcx:512 * tcx + 512],
                        start=(db == 0), stop=(db == 7))
                nc.vector.tensor_scalar(
                    kt_sb[:, 2048 * e + 512 * tcx:2048 * e + 512 * tcx + 512],
                    ps[:], bk_sb[:, e:e + 1], None, add)

            def VU(tb, pr):
                # V rows for token block tb, features 256*pr:256*pr+256
                ps = pp.tile([128, 512], f32, tag="pp", name=f"v{tb}{pr}")
                for db in range(8):
                    nc.tensor.matmul(
                        ps[:, 0:256],
                        lhsT=xv[db][:, 128 * tb:128 * tb + 128],
                        rhs=wv_sb[:, 512 * db + 256 * pr:512 * db + 256 * pr + 256],
                        start=(db == 0), stop=(db == 7))
                nc.vector.tensor_copy(
                    vaug_v[:, tb, 4 * pr:4 * pr + 4, 0:64],
                    ps[:, 0:256].rearrange("p (h c) -> p h c", h=4, c=64))

            def OU(t, i, xts):
                # output projection for token block 8*t + i
                ps = pp.tile([128, 512], f32, tag="pp", name=f"o{t}{i}")
                for idx in range(8):
                    nc.tensor.matmul(
                        ps[:],
                        lhsT=xts[idx][:, 128 * i:128 * i + 128],
                        rhs=wo_sb[:, 512 * idx:512 * idx + 512],
                        start=(idx == 0), stop=(idx == 7))
                ysb = ysbp.tile([128, 512], f32, tag="ysb", name=f"ysb{t}{i}")
                nc.vector.tensor_copy(ysb[:], ps[:])
                tb = 8 * t + i
                nc.sync.dma_start(y[128 * tb:128 * tb + 128, :], ysb[:])

            # ---- prologue: tensor work while the x stream lands ----
            for tcx in range(4):
                QU(0, tcx)
            for (e, tcx) in [(1, 0), (1, 1), (1, 2), (1, 3), (2, 0), (2, 1)]:
                QU(e, tcx)
            for tb in range(NKB):
                VU(tb, 0)
            # deferred wo DMA triggers (gpsimd queue is free now)
            for db in range(8):
                nc.gpsimd.dma_start(wo_sb[:, 512 * db:512 * db + 512],
                                    woT[128 * db:128 * db + 128, :])
            for tcx in range(4):
                KU(0, tcx)

            # filler thunks per head-pair block
            fillers = {
                0: [lambda e=e, t=t: QU(e, t)
                    for (e, t) in [(2, 2), (2, 3), (3, 0), (3, 1), (3, 2), (3, 3)]]
                   + [lambda t=t: KU(1, t) for t in range(4)],
                1: [lambda t=t: KU(2, t) for t in range(4)]
                   + [lambda tb=tb: VU(tb, 1) for tb in range(NKB)]
                   + [lambda t=t: KU(3, t) for t in range(4)],
                2: [],
                3: [],  # filled dynamically with OU(0, *) after the t0 gather
            }

            xt = {0: [], 1: []}

            def load_xt(t, cis):
                for ci in cis:
                    for r2 in range(2):
                        x = xin.tile([128, N], bf16, tag="xin",
                                     name=f"xt{t}_{ci}_{r2}")
                        nc.gpsimd.dma_start(
                            x[:, 0:1024],
                            cc_out[ci][t][128 * r2:128 * r2 + 128, :])
                        xt[t].append(x)

            # ---- attention streams ----
            def emit_group(e, c, gi):
                hb = 2048 * e
                js = [2 * gi, 2 * gi + 1]
                SG = sgp.tile([128, 4 * QC], f32, tag="SG",
                              name=f"SG{e}_{c}_{gi}")
                for m, j in enumerate(js):
                    for half in (0, 1):
                        po = 64 * half
                        off = 512 * half + QC * m
                        kt_j = kt_sb[po:po + 64,
                                     hb + 128 * j:hb + 128 * j + 128]
                        if j <= 2 * c:
                            nc.tensor.matmul(
                                SG[:, off:off + QC], lhsT=kt_j,
                                rhs=qt_sb[po:po + 64,
                                          hb + QC * c:hb + QC * c + QC],
                                start=True, stop=True,
                                skip_group_check=True)
                        else:  # j == 2c+1: front half is dead
                            nc.tensor.matmul(
                                SG[:, off + 128:off + QC],
                                lhsT=kt_j,
                                rhs=qt_sb[po:po + 64,
                                          hb + QC * c + 128:
                                          hb + QC * c + QC],
                                start=True, stop=True,
                                skip_group_check=True)
                PT = ptp.tile([128, 4 * QC], bf16, tag="PT",
                              name=f"PT{e}_{c}_{gi}")
                nc.scalar.activation(PT[:, :], SG[:, :], Exp, scale=0.125)
                if js[-1] == 2 * c + 1:  # band group: mask diagonal blocks
                    for half in (0, 1):
                        off = 512 * half
                        nc.vector.tensor_tensor(
                            PT[:, off:off + 128],
                            PT[:, off:off + 128], tri_sb[:], mult)
                        nc.vector.tensor_tensor(
                            PT[:, off + QC + 128:off + 2 * QC],
                            PT[:, off + QC + 128:off + 2 * QC],
                            tri_sb[:], mult)
                return (c, js, PT)

            ots_by_chunk = {}

            def emit_pv(e, prev):
                pc, pjs, pPT = prev
                if pjs[0] == 0:
                    OT = otp.tile([128, 2 * QC], f32, tag="OT",
                                  name=f"OT{e}_{pc}")
                    ots_by_chunk[pc] = OT
                OT = ots_by_chunk[pc]
                for m, j in enumerate(pjs):
                    for half in (0, 1):
                        h = 2 * e + half
                        va = vaug_v[:, j, h, 0:128]
                        if j <= 2 * pc:
                            nc.tensor.matmul(
                                OT[:, QC * half:QC * half + QC],
                                lhsT=va,
                                rhs=pPT[:, 512 * half + QC * m:
                                        512 * half + QC * m + QC],
                                # start clears has_written bank-wide: only
                                # the tile's very first matmul may carry it
                                start=(j == 0 and half == 0),
                                stop=(j == 2 * pc + 1),
                                skip_group_check=True)
                        else:  # j == 2pc+1: stream only the live back half
                            nc.tensor.matmul(
                                OT[:, QC * half + 128:QC * half + QC],
                                lhsT=va,
                                rhs=pPT[:, 512 * half + QC * m + 128:
                                        512 * half + QC * m + QC],
                                start=False,
                                stop=(j == 2 * pc + 1),
                                skip_group_check=True)

            def emit_epilogue(e, pc, is_last):
                OT = ots_by_chunk.pop(pc)
                t = pc // 4
                xcol = 1024 * t + QC * (pc % 4)
                # custom-DVE recip mislowers base-partition-64 operands:
                # stage the replicated d rows at base 0 first
                dtm = bcsp.tile([64, 2 * QC], f32, tag="bcs",
                                name=f"dtm{e}_{pc}")
                nc.vector.tensor_copy(dtm[:, :], OT[64:128, :])
                bcs = bcsp.tile([64, 2 * QC], f32, tag="bcs",
                                name=f"bcs{e}_{pc}")
                nc.vector.reciprocal_approx_fast(bcs[:, :], dtm[:, :])
                for half in (0, 1):
                    nc.vector.tensor_tensor(
                        xtown[64 * half:64 * half + 64, xcol:xcol + QC],
                        OT[0:64, QC * half:QC * half + QC],
                        bcs[:, QC * half:QC * half + QC], mult)
                if pc in (3, 7):
                    if e == 0 and pc == 3:
                        nc.sync.dma_start(dbg[:, :], xtown[:, 0:1024])
                    nc.sync.dma_start(cc_in[e][t][:],
                                      xtown[:, 1024 * t:1024 * t + 1024])
                    nc.gpsimd.collective_compute(
                        "AllGather",
                        mybir.AluOpType.bypass,
                        replica_groups=[[0, 1], [2, 3], [4, 5], [6, 7]],
                        ins=[cc_in[e][t].opt()],
                        outs=[cc_out[e][t].opt()],
                    )
                    if e == 3 and t == 0:
                        # final t0 gather: load its xt tiles and queue the
                        # t0 output projection as filler for chunks 4-7
                        load_xt(0, [3])
                        fillers[3].extend(
                            [lambda i=i: OU(0, i, xt[0]) for i in range(8)])

            def attention_stream(e):
                groups = [(c, gi) for c in range(8) for gi in range(c + 1)]
                fl = fillers[e]
                fill_idx = 0
                prev = None
                for idx in range(len(groups) + 1):
                    cur = None
                    if idx < len(groups):
                        c, gi = groups[idx]
                        cur = emit_group(e, c, gi)
                    if prev is not None:
                        emit_pv(e, prev)
                        pc, pjs, _ = prev
                        if pjs[-1] == 2 * pc + 1:
                            emit_epilogue(e, pc, e == 3 and pc == 7)
                    # weave fillers evenly across the group stream
                    # (fillers[3] grows mid-stream after the t0 gather)
                    target = (len(fl) * (idx + 1)) // (len(groups) + 1)
                    if e == 3:
                        target = max(0, min(len(fl), (idx + 1 - 22) * 2))
                    while fill_idx < min(target, len(fl)):
                        fl[fill_idx]()
                        fill_idx += 1
                    prev = cur
                while fill_idx < len(fl):
                    fl[fill_idx]()
                    fill_idx += 1

            attention_stream(0)
            attention_stream(1)
            attention_stream(2)
            # xt tiles for the t0 out-projection (gathers for e0..e2 are
            # long done; reuses xq slots, free after the Q fillers)
            load_xt(0, [0, 1, 2])
            # t1 xt tiles for e0..e2 (their t1 gathers fired at chunk 7)
            load_xt(1, [0, 1, 2])
            attention_stream(3)

            # ---- tail: final gather -> last xt pair -> t1 out-proj ----
            load_xt(1, [3])
            for i in range(8):
                OU(1, i, xt[1])

    nc.compile()
    return nc


def _program():
    global _PROG
    if _PROG is None:
        _PROG = _build_program()
    return _PROG


def _host_inputs(q, k, v, Wq, bq, Wk, bk, Wv, bv, Wo):
    qb = np.asarray(q, np.float32).astype(BF16)
    kb = np.asarray(k, np.float32).astype(BF16)
    vb = np.asarray(v, np.float32).astype(BF16)
    xqT = [np.ascontiguousarray(qb[b].T) for b in range(B)]
    xkT = [np.ascontiguousarray(kb[b].T) for b in range(B)]
    xvT = [np.ascontiguousarray(vb[b].T) for b in range(B)]

    def wslice(W, g):
        return np.ascontiguousarray(
            np.asarray(W, np.float32)[FG * g:FG * (g + 1), :].T).astype(BF16)

    wqg = [wslice(Wq, g) for g in range(2)]
    wkg = [wslice(Wk, g) for g in range(2)]
    wvg = [wslice(Wv, g) for g in range(2)]

    def woslice(g):
        wt = np.ascontiguousarray(
            np.asarray(Wo, np.float32)[FG * g:FG * (g + 1), :].T).astype(BF16)
        # permute 128-row input-feature blocks to the chunked-AG order
        return np.ascontiguousarray(
            wt.reshape(8, 128, FG)[PERM].reshape(D, FG))

    wog = [woslice(g) for g in range(2)]

    def bslice(bvec, g):
        return np.ascontiguousarray(
            np.asarray(bvec, np.float32)[FG * g:FG * (g + 1)]
            .reshape(4, 128).T)

    bqg = [bslice(bq, g) for g in range(2)]
    bkg = [bslice(bk, g) for g in range(2)]

    kk, qq = np.meshgrid(np.arange(128), np.arange(128), indexing="ij")
    tri = np.where(kk <= qq, 1.0, 0.0).astype(BF16)

    in_maps = []
    for core in range(N_CORES):
        b, g = core // 2, core % 2
        in_maps.append({
            "xqT": xqT[b], "xkT": xkT[b], "xvT": xvT[b],
            "wqT": wqg[g], "wkT": wkg[g], "wvT": wvg[g], "woT": wog[g],
            "bq2": bqg[g], "bk2": bkg[g], "tri01": tri,
        })
    return in_maps


def run_sharded(in_maps, trace=False, trace_kwargs=None):
    from concourse.bass_utils import run_bass_kernel_spmd
    nc = _program()
    return run_bass_kernel_spmd(nc, in_maps, core_ids=list(range(N_CORES)),
                                trace=trace, trace_kwargs=trace_kwargs or {})


def kernel(q, k, v, Wq, bq, Wk, bk, Wv, bv, Wo):
    in_maps = _host_inputs(q, k, v, Wq, bq, Wk, bk, Wv, bv, Wo)
    res = run_sharded(in_maps)
    out = np.empty((B, N, D), np.float32)
    for b in range(B):
        out[b, :, 0:FG] = res.results[2 * b]["y"]
        out[b, :, FG:D] = res.results[2 * b + 1]["y"]
    return out


# revision 24
# speedup vs baseline: 1.1403x; 1.0274x over previous
"""Trainium2 Bass kernel for nn_MultiHeadAttention_36051955483000.

Full-shape contract: kernel(**inputs) takes the complete fp32 tensors
(q,k,v: [4,2048,1024]; Wq/Wk/Wv/Wo: [1024,1024]; biases [1024]) and
returns the full [4,2048,1024] fp32 output.

Sharding (8 NeuronCores): core = 2*b + g for batch b in 0..3 and
head-group g in {0,1}. Each core computes 8 of the 16 heads for one
batch, then a pairwise AllGather and the output projection for its 512
output features.

v3 design (single fused instruction stream):
- PE warmup matmuls at t=0 so the HAM clock gate is released before
  real work arrives.
- Input DMA order xq -> xv -> xk; V-projection units run while xk is
  still landing, so attention starts the moment kt is ready.
- The scalar engine (softmax exp over ~19M elements) paces attention,
  so projection and output-projection matmuls are woven as FILLER
  between attention score-groups, keeping the tensor engine dense.
- V is augmented with 64 ones-columns per (kblock, head): the PV
  matmul (M=128) emits softmax denominators replicated across PSUM
  partitions 64:128. Epilogue: stage d at base partition 0 (custom-DVE
  recip mislowers base-64 operands), reciprocal, two multiplies.
- Band-group dead halves are never streamed through PV, never memset.
- AllGather per (head-pair, token-half) with Shared-space outputs,
  overlapped with attention. Output projection t=0 half runs as filler
  inside the last head-pair block; only the t=1 half trails the final
  collective. tile_wait_until pins the gather-dependent loads late so
  the scheduler cannot hoist their waits ahead of collective triggers.
"""

import numpy as np
import ml_dtypes

B, N, D, H = 4, 2048, 1024, 16
DH = D // H            # 64
HG = H // 2            # 8 heads per core
FG = D // 2            # 512 features per head-group
N_CORES = 8
QC = 256               # query-chunk width
NKB = N // 128         # 16 key blocks

BF16 = ml_dtypes.bfloat16
# chunked-AllGather feature-block order (see _build_program)
PERM = [0, 4, 1, 5, 2, 6, 3, 7]

WARMUP_MMS = 44

_PROG = None


def _build_program():
    from concourse import bacc, tile, mybir

    f32 = mybir.dt.float32
    bf16 = mybir.dt.bfloat16

    nc = bacc.Bacc("TRN2", target_bir_lowering=False, debug=False,
                   num_devices=N_CORES)

    xqT = nc.dram_tensor("xqT", [D, N], bf16, kind="ExternalInput").ap()
    xkT = nc.dram_tensor("xkT", [D, N], bf16, kind="ExternalInput").ap()
    xvT = nc.dram_tensor("xvT", [D, N], bf16, kind="ExternalInput").ap()
    wqT = nc.dram_tensor("wqT", [D, FG], bf16, kind="ExternalInput").ap()
    wkT = nc.dram_tensor("wkT", [D, FG], bf16, kind="ExternalInput").ap()
    wvT = nc.dram_tensor("wvT", [D, FG], bf16, kind="ExternalInput").ap()
    woT = nc.dram_tensor("woT", [D, FG], bf16, kind="ExternalInput").ap()
    bq2 = nc.dram_tensor("bq2", [128, 4], f32, kind="ExternalInput").ap()
    bk2 = nc.dram_tensor("bk2", [128, 4], f32, kind="ExternalInput").ap()
    tri01 = nc.dram_tensor("tri01", [128, 128], bf16, kind="ExternalInput").ap()
    y = nc.dram_tensor("y", [N, FG], f32, kind="ExternalOutput").ap()

    add = mybir.AluOpType.add
    mult = mybir.AluOpType.mult
    Exp = mybir.ActivationFunctionType.Exp

    with tile.TileContext(nc) as tc:
        with (
            tc.tile_pool(name="consts", bufs=1) as consts,
            tc.tile_pool(name="dram", bufs=1, space="DRAM") as dram,
            tc.tile_pool(name="xin", bufs=24) as xin,
            tc.tile_pool(name="pt", bufs=2) as ptp,
            tc.tile_pool(name="bcs", bufs=2) as bcsp,
            tc.tile_pool(name="ysb", bufs=1) as ysbp,
            tc.tile_pool(name="sg", bufs=2, space="PSUM") as sgp,
            tc.tile_pool(name="ot", bufs=2, space="PSUM") as otp,
            tc.tile_pool(name="pp", bufs=2, space="PSUM") as pp,
        ):
            wq_sb = consts.tile([128, 8 * FG], bf16, tag="wq")
            wk_sb = consts.tile([128, 8 * FG], bf16, tag="wk")
            wv_sb = consts.tile([128, 8 * FG], bf16, tag="wv")
            wo_sb = consts.tile([128, 8 * FG], bf16, tag="wo")
            qt_sb = consts.tile([128, 4 * N], bf16, tag="qt")
            kt_sb = consts.tile([128, 4 * N], bf16, tag="kt")
            # per (kblock, head): cols 0:64 = V^T block, 64:128 = ones
            vaug = consts.tile([128, NKB * HG * 128], bf16, tag="vaug")
            xtown = consts.tile([128, 2048], bf16, tag="xtown")
            warm_r = consts.tile([128, 512], bf16, tag="warm")
            bq_sb = consts.tile([128, 4], f32, tag="bq")
            bk_sb = consts.tile([128, 4], f32, tag="bk")
            tri_sb = consts.tile([128, 128], bf16, tag="tri")

            vaug_v = vaug[:, :].rearrange("p (t h c) -> p t h c",
                                          t=NKB, h=HG, c=128)

            cc_in = [[dram.tile([128, N // 2], bf16, name=f"cc_in{e}_{t}",
                                tag=f"cci{e}_{t}") for t in range(2)]
                     for e in range(4)]
            cc_out = [[dram.tile([256, N // 2], bf16, name=f"cc_out{e}_{t}",
                                 tag=f"cco{e}_{t}") for t in range(2)]
                      for e in range(4)]

            # ---- DMA triggers ----
            nc.gpsimd.dma_start(bq_sb[:], bq2[:])
            nc.gpsimd.dma_start(bk_sb[:], bk2[:])
            nc.gpsimd.dma_start(tri_sb[:], tri01[:])
            for db in range(8):
                nc.gpsimd.dma_start(wv_sb[:, 512 * db:512 * db + 512],
                                    wvT[128 * db:128 * db + 128, :])
            # scalar queue (idle until exp starts): wq, wk
            for db in range(8):
                nc.scalar.dma_start(wq_sb[:, 512 * db:512 * db + 512],
                                    wqT[128 * db:128 * db + 128, :])
            for db in range(8):
                nc.scalar.dma_start(wk_sb[:, 512 * db:512 * db + 512],
                                    wkT[128 * db:128 * db + 128, :])
            # sync queue: the big x stream in need-order xq, xv, xk
            xq = [xin.tile([128, N], bf16, tag="xin", name=f"xq{db}")
                  for db in range(8)]
            for db in range(8):
                nc.sync.dma_start(xq[db][:], xqT[128 * db:128 * db + 128, :])
            xv = [xin.tile([128, N], bf16, tag="xin", name=f"xv{db}")
                  for db in range(8)]
            for db in range(8):
                nc.sync.dma_start(xv[db][:], xvT[128 * db:128 * db + 128, :])
            xk = [xin.tile([128, N], bf16, tag="xin", name=f"xk{db}")
                  for db in range(8)]
            for db in range(8):
                nc.sync.dma_start(xk[db][:], xkT[128 * db:128 * db + 128, :])

            nc.vector.memset(warm_r[:, :], 1.0)
            nc.vector.memset(vaug_v[:, :, :, 64:128], 1.0)

            # ---- PE warmup: release the HAM clock gate before real work
            wps = pp.tile([128, 512], f32, tag="pp", name="warm")
            for _ in range(WARMUP_MMS):
                nc.tensor.matmul(wps[:], lhsT=warm_r[:, 0:128], rhs=warm_r[:],
                                 start=True, stop=True)

            # ---- unit emitters ----
            def QU(e, tcx):
                ps = pp.tile([128, 512], f32, tag="pp", name=f"q{e}{tcx}")
                for db in range(8):
                    nc.tensor.matmul(
                        ps[:],
                        lhsT=wq_sb[:, 512 * db + 128 * e:512 * db + 128 * e + 128],
                        rhs=xq[db][:, 512 * tcx:512 * tcx + 512],
                        start=(db == 0), stop=(db == 7))
                nc.vector.tensor_scalar(
                    qt_sb[:, 2048 * e + 512 * tcx:2048 * e + 512 * tcx + 512],
                    ps[:], bq_sb[:, e:e + 1], None, add)

            def KU(e, tcx):
                ps = pp.tile([128, 512], f32, tag="pp", name=f"k{e}{tcx}")
                for db in range(8):
                    nc.tensor.matmul(
                        ps[:],
                        lhsT=wk_sb[:, 512 * db + 128 * e:512 * db + 128 * e + 128],
                        rhs=xk[db][:, 512 * tcx:512 * tcx + 512],
                        start=(db == 0), stop=(db == 7))
                nc.vector.tensor_scalar(
                    kt_sb[:, 2048 * e + 512 * tcx:2048 * e + 512 * tcx + 512],
                    ps[:], bk_sb[:, e:e + 1], None, add)

            def VU(tb):
                # V rows for token block tb, all 8 heads
                ps = pp.tile([128, 512], f32, tag="pp", name=f"v{tb}")
                for db in range(8):
                    nc.tensor.matmul(
                        ps[:],
                        lhsT=xv[db][:, 128 * tb:128 * tb + 128],
                        rhs=wv_sb[:, 512 * db:512 * db + 512],
                        start=(db == 0), stop=(db == 7))
                nc.vector.tensor_copy(
                    vaug_v[:, tb, :, 0:64],
                    ps[:, :].rearrange("p (h c) -> p h c", h=8, c=64))

            def OU(t, i, xts):
                # output projection for token block 8*t + i
                ps = pp.tile([128, 512], f32, tag="pp", name=f"o{t}{i}")
                for idx in range(8):
                    nc.tensor.matmul(
                        ps[:],
                        lhsT=xts[idx][:, 128 * i:128 * i + 128],
                        rhs=wo_sb[:, 512 * idx:512 * idx + 512],
                        start=(idx == 0), stop=(idx == 7))
                ysb = ysbp.tile([128, 512], f32, tag="ysb", name=f"ysb{t}{i}")
                nc.vector.tensor_copy(ysb[:], ps[:])
                tb = 8 * t + i
                nc.sync.dma_start(y[128 * tb:128 * tb + 128, :], ysb[:])

            # ---- prologue: tensor work while the x stream lands ----
            for tcx in range(4):
                QU(0, tcx)
            for tcx in range(4):
                QU(1, tcx)
            for tcx in range(4):
                QU(2, tcx)
            for tb in range(6):
                VU(tb)
            # deferred wo DMA triggers (gpsimd queue is free now)
            for db in range(8):
                nc.gpsimd.dma_start(wo_sb[:, 512 * db:512 * db + 512],
                                    woT[128 * db:128 * db + 128, :])
            for tcx in range(4):
                KU(0, tcx)

            # filler thunks per head-pair block
            fillers = {
                0: [lambda tb=tb: VU(tb) for tb in range(6, NKB)]
                   + [lambda t=t: KU(1, t) for t in range(4)],
                1: [lambda t=t: KU(2, t) for t in range(4)],
                2: [lambda t=t: QU(3, t) for t in range(4)]
                   + [lambda t=t: KU(3, t) for t in range(4)],
                3: [],  # filled dynamically with OU(0, *) after the t0 gather
            }

            xt = {0: [], 1: []}

            def load_xt(t, cis):
                for ci in cis:
                    for r2 in range(2):
                        x = xin.tile([128, N], bf16, tag="xin",
                                     name=f"xt{t}_{ci}_{r2}")
                        nc.gpsimd.dma_start(
                            x[:, 0:1024],
                            cc_out[ci][t][128 * r2:128 * r2 + 128, :])
                        xt[t].append(x)

            # ---- attention streams ----
            def emit_group(e, c, gi):
                hb = 2048 * e
                js = [2 * gi, 2 * gi + 1]
                SG = sgp.tile([128, 4 * QC], f32, tag="SG",
                              name=f"SG{e}_{c}_{gi}")
                for m, j in enumerate(js):
                    for half in (0, 1):
                        po = 64 * half
                        off = 512 * half + QC * m
                        kt_j = kt_sb[po:po + 64,
                                     hb + 128 * j:hb + 128 * j + 128]
                        if j <= 2 * c:
                            nc.tensor.matmul(
                                SG[:, off:off + QC], lhsT=kt_j,
                                rhs=qt_sb[po:po + 64,
                                          hb + QC * c:hb + QC * c + QC],
                                start=True, stop=True,
                                skip_group_check=True)
                        else:  # j == 2c+1: front half is dead
                            nc.tensor.matmul(
                                SG[:, off + 128:off + QC],
                                lhsT=kt_j,
                                rhs=qt_sb[po:po + 64,
                                          hb + QC * c + 128:
                                          hb + QC * c + QC],
                                start=True, stop=True,
                                skip_group_check=True)
                PT = ptp.tile([128, 4 * QC], bf16, tag="PT",
                              name=f"PT{e}_{c}_{gi}")
                nc.scalar.activation(PT[:, :], SG[:, :], Exp, scale=0.125)
                if js[-1] == 2 * c + 1:  # band group: mask diagonal blocks
                    for half in (0, 1):
                        off = 512 * half
                        nc.vector.tensor_tensor(
                            PT[:, off:off + 128],
                            PT[:, off:off + 128], tri_sb[:], mult)
                        nc.vector.tensor_tensor(
                            PT[:, off + QC + 128:off + 2 * QC],
                            PT[:, off + QC + 128:off + 2 * QC],
                            tri_sb[:], mult)
                return (c, js, PT)

            ots_by_chunk = {}

            def emit_pv(e, prev):
                pc, pjs, pPT = prev
                if pjs[0] == 0:
                    OT = otp.tile([128, 2 * QC], f32, tag="OT",
                                  name=f"OT{e}_{pc}")
                    ots_by_chunk[pc] = OT
                OT = ots_by_chunk[pc]
                for m, j in enumerate(pjs):
                    for half in (0, 1):
                        h = 2 * e + half
                        va = vaug_v[:, j, h, 0:128]
                        if j <= 2 * pc:
                            nc.tensor.matmul(
                                OT[:, QC * half:QC * half + QC],
                                lhsT=va,
                                rhs=pPT[:, 512 * half + QC * m:
                                        512 * half + QC * m + QC],
                                # start clears has_written bank-wide: only
                                # the tile's very first matmul may carry it
                                start=(j == 0 and half == 0),
                                stop=(j == 2 * pc + 1),
                                skip_group_check=True)
                        else:  # j == 2pc+1: stream only the live back half
                            nc.tensor.matmul(
                                OT[:, QC * half + 128:QC * half + QC],
                                lhsT=va,
                                rhs=pPT[:, 512 * half + QC * m + 128:
                                        512 * half + QC * m + QC],
                                start=False,
                                stop=(j == 2 * pc + 1),
                                skip_group_check=True)

            def emit_epilogue(e, pc):
                OT = ots_by_chunk.pop(pc)
                t = pc // 4
                xcol = 1024 * t + QC * (pc % 4)
                # custom-DVE recip mislowers base-partition-64 operands:
                # stage the replicated d rows at base 0 first
                dtm = bcsp.tile([64, 2 * QC], f32, tag="bcs",
                                name=f"dtm{e}_{pc}")
                nc.vector.tensor_copy(dtm[:, :], OT[64:128, :])
                bcs = bcsp.tile([64, 2 * QC], f32, tag="bcs",
                                name=f"bcs{e}_{pc}")
                nc.vector.reciprocal_approx_fast(bcs[:, :], dtm[:, :])
                for half in (0, 1):
                    nc.vector.tensor_tensor(
                        xtown[64 * half:64 * half + 64, xcol:xcol + QC],
                        OT[0:64, QC * half:QC * half + QC],
                        bcs[:, QC * half:QC * half + QC], mult)
                if pc in (3, 7):
                    nc.sync.dma_start(cc_in[e][t][:],
                                      xtown[:, 1024 * t:1024 * t + 1024])
                    nc.gpsimd.collective_compute(
                        "AllGather",
                        mybir.AluOpType.bypass,
                        replica_groups=[[0, 1], [2, 3], [4, 5], [6, 7]],
                        ins=[cc_in[e][t].opt()],
                        outs=[cc_out[e][t].opt()],
                    )
                    if e == 3 and t == 0:
                        # final t0 gather: its xt pair + the t0 out-proj
                        # queue as filler for chunks 4-7
                        with tc.tile_wait_until(0.215):
                            load_xt(0, [3])
                        fillers[3].extend(
                            [lambda i=i: OU(0, i, xt[0]) for i in range(8)])

            def attention_stream(e):
                groups = [(c, gi) for c in range(8) for gi in range(c + 1)]
                fl = fillers[e]
                fill_idx = 0
                prev = None
                for idx in range(len(groups) + 1):
                    cur = None
                    if idx < len(groups):
                        c, gi = groups[idx]
                        cur = emit_group(e, c, gi)
                    if prev is not None:
                        emit_pv(e, prev)
                        pc, pjs, _ = prev
                        if pjs[-1] == 2 * pc + 1:
                            emit_epilogue(e, pc)
                    # weave fillers evenly across the group stream
                    # (fillers[3] grows mid-stream after the t0 gather)
                    target = (len(fl) * (idx + 1)) // (len(groups) + 1)
                    if e == 3:
                        target = max(0, min(len(fl), (idx + 1 - 14) * 2))
                    while fill_idx < min(target, len(fl)):
                        fl[fill_idx]()
                        fill_idx += 1
                    prev = cur
                while fill_idx < len(fl):
                    fl[fill_idx]()
                    fill_idx += 1

            attention_stream(0)
            attention_stream(1)
            attention_stream(2)
            # gather-dependent loads for the out-projection: all gathers for
            # e0..e2 are done by e3; pin them at e3 logical time so the
            # scheduler cannot hoist their waits ahead of e3's collectives
            with tc.tile_wait_until(0.195):
                load_xt(0, [0, 1, 2])
                load_xt(1, [0, 1, 2])
            attention_stream(3)

            # ---- tail: final gather -> last xt pair -> t1 out-proj ----
            with tc.tile_wait_until(0.235):
                load_xt(1, [3])
                for i in range(8):
                    OU(1, i, xt[1])

    nc.compile()
    return nc


def _program():
    global _PROG
    if _PROG is None:
        _PROG = _build_program()
    return _PROG


def _host_inputs(q, k, v, Wq, bq, Wk, bk, Wv, bv, Wo):
    qb = np.asarray(q, np.float32).astype(BF16)
    kb = np.asarray(k, np.float32).astype(BF16)
    vb = np.asarray(v, np.float32).astype(BF16)
    xqT = [np.ascontiguousarray(qb[b].T) for b in range(B)]
    xkT = [np.ascontiguousarray(kb[b].T) for b in range(B)]
    xvT = [np.ascontiguousarray(vb[b].T) for b in range(B)]

    def wslice(W, g):
        return np.ascontiguousarray(
            np.asarray(W, np.float32)[FG * g:FG * (g + 1), :].T).astype(BF16)

    wqg = [wslice(Wq, g) for g in range(2)]
    wkg = [wslice(Wk, g) for g in range(2)]
    wvg = [wslice(Wv, g) for g in range(2)]

    def woslice(g):
        wt = np.ascontiguousarray(
            np.asarray(Wo, np.float32)[FG * g:FG * (g + 1), :].T).astype(BF16)
        # permute 128-row input-feature blocks to the chunked-AG order
        return np.ascontiguousarray(
            wt.reshape(8, 128, FG)[PERM].reshape(D, FG))

    wog = [woslice(g) for g in range(2)]

    def bslice(bvec, g):
        return np.ascontiguousarray(
            np.asarray(bvec, np.float32)[FG * g:FG * (g + 1)]
            .reshape(4, 128).T)

    bqg = [bslice(bq, g) for g in range(2)]
    bkg = [bslice(bk, g) for g in range(2)]

    kk, qq = np.meshgrid(np.arange(128), np.arange(128), indexing="ij")
    tri = np.where(kk <= qq, 1.0, 0.0).astype(BF16)

    in_maps = []
    for core in range(N_CORES):
        b, g = core // 2, core % 2
        in_maps.append({
            "xqT": xqT[b], "xkT": xkT[b], "xvT": xvT[b],
            "wqT": wqg[g], "wkT": wkg[g], "wvT": wvg[g], "woT": wog[g],
            "bq2": bqg[g], "bk2": bkg[g], "tri01": tri,
        })
    return in_maps


def run_sharded(in_maps, trace=False, trace_kwargs=None):
    from concourse.bass_utils import run_bass_kernel_spmd
    nc = _program()
    return run_bass_kernel_spmd(nc, in_maps, core_ids=list(range(N_CORES)),
                                trace=trace, trace_kwargs=trace_kwargs or {})


def kernel(q, k, v, Wq, bq, Wk, bk, Wv, bv, Wo):
    in_maps = _host_inputs(q, k, v, Wq, bq, Wk, bk, Wv, bv, Wo)
    res = run_sharded(in_maps)
    out = np.empty((B, N, D), np.float32)
    for b in range(B):
        out[b, :, 0:FG] = res.results[2 * b]["y"]
        out[b, :, FG:D] = res.results[2 * b + 1]["y"]
    return out
